# revision 50
# baseline (speedup 1.0000x reference)
"""3-layer GAT on 8 Trainium2 NeuronCores (Bass/Tile).

Edge-sharded by destination range:
  - Nodes split into 8 contiguous ranges (one per core); each core owns the
    softmax + aggregation for its destination nodes.
  - Per layer a packed per-node table [h | a_src] (c-major feature order) is
    computed locally and AllGathered (bf16, 768B rows); a_dst lives in a
    per-core local table (256B rows).
  - Edges (with self loops) are bucketed per core into 128-dst tiles x
    128-edge chunks; chunk structure (incl. lo/hi int16-index table halves)
    is made identical across cores so one SPMD instruction stream fits all.
  - Per 8-chunk super-batch the kernel dma_gathers source rows + dest
    attention rows, computes w = exp(leaky_relu(a_src+a_dst)) (softmax
    shift-invariance removes the segment-max pass at these value ranges),
    scales messages by w, and segment-sums with matmuls against one-hot
    membership matrices (tensor_scalar is_equal vs an iota tile), keeping
    numerator and denominator together in PSUM.  The per-tile epilogue
    divides, applies bias/relu, and feeds the next layer's matmul whose rhs
    [W | W@att_src | W@att_dst] also emits the next attention scores.
"""

import numpy as np
import ml_dtypes

N = 50000
E = 800000
IN_C = 128
HID = 32
OUT_C = 40
HEADS = 8
NEG_SLOPE = 0.2
NCORES = 8

_BF16 = ml_dtypes.bfloat16

KSUP = 8  # chunks per gather super-batch (1024 idx = dma_gather limit)


def _cmajor_perm(heads, ch):
    f_new = np.arange(heads * ch)
    return (f_new % heads) * ch + f_new // heads  # perm[new] = old


def _attn_cols(w, att):
    heads, ch = att.shape
    return np.einsum("khc,hc->kh", w.reshape(-1, heads, ch), att).astype(np.float32)


def _prep_weights(W1, as1, ad1, b1, W2, as2, ad2, b2, W3, as3, ad3, b3):
    W1 = np.asarray(W1, np.float32)
    W2 = np.asarray(W2, np.float32)
    W3 = np.asarray(W3, np.float32)
    perm = _cmajor_perm(HEADS, HID)

    rhs1b = np.concatenate(
        [W1[:, perm], _attn_cols(W1, np.asarray(as1, np.float32))],
        axis=1).astype(_BF16)
    rhs1a = _attn_cols(W1, np.asarray(ad1, np.float32)).astype(_BF16)
    W2r = W2[perm, :]
    rhs2 = np.concatenate(
        [W2r[:, perm], _attn_cols(W2r, np.asarray(as2, np.float32)),
         _attn_cols(W2r, np.asarray(ad2, np.float32))], axis=1).astype(np.float32)
    W3r = W3[perm, :]
    as3p = (W3r @ np.asarray(as3, np.float32)[0]).reshape(-1, 1)
    ad3p = (W3r @ np.asarray(ad3, np.float32)[0]).reshape(-1, 1)
    rhs3 = np.concatenate([W3r, as3p, ad3p], axis=1).astype(np.float32)

    def bcast(b):
        return np.tile(np.asarray(b, np.float32)[None, :], (128, 1))

    return (rhs1b, rhs1a, rhs2, rhs3,
            bcast(np.asarray(b1, np.float32)[perm]),
            bcast(np.asarray(b2, np.float32)[perm]),
            bcast(np.asarray(b3, np.float32)))


SLICES = [25, 24]  # tiles per AG slice / table half (sum must equal ntiles)


def _prep_graph(edge_index):
    """Slot edges into the SPMD-uniform (tile, section, chunk) grid.

    Table rows use a slice-major layout: row(core c, local row r) =
    base[s] + c*rs[s] + (r - 128*t0[s]) with s = slice of tile r//128, so
    each AllGather slice lands in one contiguous block of the table.
    """
    src = np.concatenate([edge_index[0], np.arange(N)]).astype(np.int64)
    dst = np.concatenate([edge_index[1], np.arange(N)]).astype(np.int64)

    npc = N // NCORES
    ntiles = (npc + 127) // 128
    nmax = ntiles * 128
    assert sum(SLICES) == ntiles

    sl_t0 = np.concatenate([[0], np.cumsum(SLICES)[:-1]])
    slice_of_tile = np.repeat(np.arange(len(SLICES)), SLICES)
    sl_rows = 128 * np.asarray(SLICES)
    sl_base = np.zeros(len(SLICES), np.int64)
    sl_base[1:] = np.cumsum(NCORES * sl_rows)[:-1]
    half = int(sl_base[1])  # 25600 and 24576 rows: both fit int16

    core_of = dst // npc
    d_loc = dst - core_of * npc
    tile_of = d_loc // 128
    s_core = src // npc
    r_loc = src - s_core * npc
    t_src = r_loc // 128
    s_sl = slice_of_tile[t_src]
    s_row = sl_base[s_sl] + s_core * sl_rows[s_sl] + (r_loc - 128 * sl_t0[s_sl])
    is_hi = s_row >= half

    cnt = np.zeros((NCORES, ntiles, 2), np.int64)
    np.add.at(cnt, (core_of, tile_of, is_hi.astype(np.int64)), 1)
    sec_cpt = np.ceil(cnt / 128).astype(np.int64).max(axis=0)  # [ntiles, 2]
    sec_cpt[:, 0] = np.maximum(sec_cpt[:, 0], 1)

    total = int(sec_cpt.sum())
    pad = (-total) % KSUP
    sec_cpt[-1, 1] += pad
    total += pad
    nsup = total // KSUP

    tile_of_chunk = []
    tag_of_chunk = []
    for t in range(ntiles):
        tile_of_chunk += [t] * int(sec_cpt[t, 0] + sec_cpt[t, 1])
        tag_of_chunk += [0] * int(sec_cpt[t, 0]) + [1] * int(sec_cpt[t, 1])
    tile_of_chunk = np.array(tile_of_chunk)
    tag_of_chunk = np.array(tag_of_chunk)
    sec_base = np.zeros((ntiles, 2), np.int64)
    sec_base.ravel()[1:] = np.cumsum(sec_cpt.ravel())[:-1]

    src_w = np.zeros((NCORES, 128, total * 8), np.int16)
    adst_w = np.zeros((NCORES, 128, total * 8), np.int16)
    seg = np.full((NCORES, nsup, 128, KSUP), 255.0, np.float32)

    order = np.lexsort((src, is_hi, tile_of, core_of))
    src_o = s_row[order]
    dst_o = d_loc[order]
    core_o = core_of[order]
    tile_o = tile_of[order]
    hi_o = is_hi[order]

    for k in range(NCORES):
        m = core_o == k
        t = tile_o[m]
        hi = hi_o[m].astype(np.int64)
        sr = src_o[m] - hi * half
        dl = dst_o[m]
        key = t * 2 + hi
        cnts = np.bincount(key, minlength=ntiles * 2)
        st = np.zeros(ntiles * 2, np.int64)
        st[1:] = np.cumsum(cnts)[:-1]
        pos = np.arange(len(t)) - st[key]
        q = sec_base[t, hi] + pos // 128
        p = pos % 128
        col = q * 8 + p // 16
        row = p % 16
        for c in range(8):
            src_w[k, row + 16 * c, col] = sr
            adst_w[k, row + 16 * c, col] = dl
        seg[k, q // KSUP, p, q % KSUP] = (dl % 128).astype(np.float32)

    runs = []  # (sup, chunk_lo, chunk_hi, tag)
    for s in range(nsup):
        q0 = s * KSUP
        r0 = q0
        for q in range(q0 + 1, q0 + KSUP + 1):
            if q == q0 + KSUP or tag_of_chunk[q] != tag_of_chunk[r0]:
                runs.append((s, r0, q, int(tag_of_chunk[r0])))
                r0 = q

    return dict(
        src_w=src_w, adst_w=adst_w, seg=seg,
        tile_of_chunk=tile_of_chunk, runs=runs, nsup=nsup, total=total,
        ntiles=ntiles, nmax=nmax, npc=npc, half=half,
    )


def _build_bass(g, repeat=1):
    import concourse.bacc as bacc
    import concourse.mybir as mybir
    import concourse.tile as tile
    from concourse.masks import make_identity

    dt = mybir.dt
    Alu = mybir.AluOpType
    Act = mybir.ActivationFunctionType

    ntiles, nmax, nsup, total = g["ntiles"], g["nmax"], g["nsup"], g["total"]
    half = g["half"]
    tile_of_chunk = g["tile_of_chunk"]
    H2 = HEADS * HID  # 256
    PACK = H2 + 2 * HEADS  # 272 psum width: h + a_src + a_dst
    TWB = 512  # layer-1/2 table row bytes: 256 fp8 h + 8 bf16 a_src + pad
    ROWB = H2 + 2 * HEADS  # 272 written bytes per row
    TW3 = 128  # layer-3 / a_dst table row width (256B)
    GW = H2 + HEADS  # 264 useful gathered cols
    GW3 = OUT_C + 1  # 41

    first_chunk = {}
    last_chunk = {}
    for q, t in enumerate(tile_of_chunk):
        first_chunk.setdefault(int(t), q)
        last_chunk[int(t)] = q
    runs_by_sup = {}
    for (s, a, b, tag) in g["runs"]:
        runs_by_sup.setdefault(s, []).append((a, b, tag))

    nc = bacc.Bacc("TRN2", target_bir_lowering=False, debug=False,
                   num_devices=NCORES, num_swdge_queues=4,
                   dynamic_dma_scratch_size=49152)

    NT = NCORES * nmax
    xTf = nc.dram_tensor("xTf", [IN_C, NT], dt.bfloat16, kind="ExternalInput")
    xTb = nc.dram_tensor("xTb", [IN_C, nmax], dt.bfloat16,
                         kind="ExternalInput")
    rhs1b = nc.dram_tensor("rhs1b", [IN_C, GW], dt.bfloat16,
                           kind="ExternalInput")
    rhs1a = nc.dram_tensor("rhs1a", [IN_C, HEADS], dt.bfloat16,
                           kind="ExternalInput")
    rhs2 = nc.dram_tensor("rhs2", [H2, PACK], dt.float32, kind="ExternalInput")
    rhs3 = nc.dram_tensor("rhs3", [H2, OUT_C + 2], dt.float32,
                          kind="ExternalInput")
    b1r = nc.dram_tensor("b1r", [128, H2], dt.float32, kind="ExternalInput")
    b2r = nc.dram_tensor("b2r", [128, H2], dt.float32, kind="ExternalInput")
    b3r = nc.dram_tensor("b3r", [128, OUT_C], dt.float32, kind="ExternalInput")
    iota = nc.dram_tensor("iota", [128, 128], dt.bfloat16, kind="ExternalInput")
    src_w = nc.dram_tensor("src_w", [128, total * 8], dt.int16,
                           kind="ExternalInput")
    adst_w = nc.dram_tensor("adst_w", [128, total * 8], dt.int16,
                            kind="ExternalInput")
    seg_in = nc.dram_tensor("seg", [nsup, 128, KSUP], dt.float32,
                            kind="ExternalInput")
    out = nc.dram_tensor("out", [nmax, OUT_C], dt.float32,
                         kind="ExternalOutput")


    with tile.TileContext(nc) as tc:
        with (
            tc.tile_pool(name="const", bufs=1) as constp,
            tc.tile_pool(name="sbuf", bufs=4) as sbuf,
            tc.tile_pool(name="gbuf", bufs=4) as gbuf,
            tc.tile_pool(name="mbuf", bufs=4) as mbuf,
            tc.tile_pool(name="epil", bufs=2) as epil,
            tc.tile_pool(name="psum_seg", bufs=3, space="PSUM") as psum_seg,
            tc.tile_pool(name="psum_h", bufs=2, space="PSUM") as psum_h,
            tc.tile_pool(name="psum_tp", bufs=2, space="PSUM") as psum_tp,
            tc.tile_pool(name="dram", bufs=1, space="DRAM") as dram,
        ):
            # ---- constants ----
            rhs1b_s = constp.tile([IN_C, GW], dt.bfloat16)
            nc.sync.dma_start(out=rhs1b_s[:], in_=rhs1b[:])
            rhs1a_s = constp.tile([IN_C, HEADS], dt.bfloat16)
            nc.sync.dma_start(out=rhs1a_s[:], in_=rhs1a[:])
            rhs2_s = constp.tile([128, 2 * PACK], dt.float32)
            nc.sync.dma_start(
                out=rhs2_s[:].rearrange("p (k f) -> p k f", k=2),
                in_=rhs2[:].rearrange("(k p) f -> p k f", p=128))
            rhs3_s = constp.tile([128, 2 * (OUT_C + 2)], dt.float32)
            nc.sync.dma_start(
                out=rhs3_s[:].rearrange("p (k f) -> p k f", k=2),
                in_=rhs3[:].rearrange("(k p) f -> p k f", p=128))
            b1_s = constp.tile([128, H2], dt.float32)
            nc.sync.dma_start(out=b1_s[:], in_=b1r[:])
            b2_s = constp.tile([128, H2], dt.float32)
            nc.sync.dma_start(out=b2_s[:], in_=b2r[:])
            b3_s = constp.tile([128, OUT_C], dt.float32)
            nc.sync.dma_start(out=b3_s[:], in_=b3r[:])
            iota_s = constp.tile([128, 128], dt.bfloat16)
            nc.sync.dma_start(out=iota_s[:], in_=iota[:])
            sidx_all = constp.tile([128, total * 8], dt.int16)
            nc.sync.dma_start(out=sidx_all[:], in_=src_w[:])
            didx_all = constp.tile([128, total * 8], dt.int16)
            nc.sync.dma_start(out=didx_all[:], in_=adst_w[:])
            seg_all = constp.tile([128, nsup * KSUP], dt.float32)
            nc.sync.dma_start(
                out=seg_all[:].rearrange("p (s k) -> p s k", k=KSUP),
                in_=seg_in[:].rearrange("s p k -> p s k"))
            ident = constp.tile([128, 128], dt.float32)
            make_identity(nc, ident[:])

            # ---- slice geometry (slice-major table rows) ----
            sl_t0 = [0]
            for n in SLICES[:-1]:
                sl_t0.append(sl_t0[-1] + n)
            slice_of_tile = []
            for s, n in enumerate(SLICES):
                slice_of_tile += [s] * n
            sl_rows = [128 * n for n in SLICES]
            sl_base = [0]
            for r in sl_rows[:-1]:
                sl_base.append(sl_base[-1] + NCORES * r)
            sl_last_tile = [t0 + n - 1 for t0, n in zip(sl_t0, SLICES)]

            # ---- DRAM temporaries ----
            tabs1 = [dram.tile([NCORES * r, TWB], dt.uint8,
                               name=f"tab1_{s}")
                     for s, r in enumerate(sl_rows)]
            loc2s = [dram.tile([r, TWB], dt.uint8, name=f"loc2_{s}")
                     for s, r in enumerate(sl_rows)]
            loc3s = [dram.tile([r, TW3], dt.bfloat16, name=f"loc3_{s}")
                     for s, r in enumerate(sl_rows)]
            adl1 = dram.tile([nmax, TW3], dt.bfloat16)
            adl2 = dram.tile([nmax, TW3], dt.bfloat16)
            adl3 = dram.tile([nmax, TW3], dt.bfloat16)

            def pack12(ps, locs, adl, t):
                s = slice_of_tile[t]
                r0 = (t - sl_t0[s]) * 128
                pk = epil.tile([128, TWB], dt.uint8, tag="pack")
                nc.scalar.copy(out=pk[:, :H2].bitcast(dt.float8e4),
                               in_=ps[:, :H2])
                nc.scalar.copy(
                    out=pk[:, H2:H2 + 2 * HEADS].bitcast(dt.bfloat16),
                    in_=ps[:, H2:H2 + HEADS])
                nc.sync.dma_start(out=locs[s][r0:r0 + 128, :ROWB],
                                  in_=pk[:, :ROWB])
                pa = epil.tile([128, HEADS], dt.bfloat16, tag="packa")
                nc.scalar.copy(out=pa[:], in_=ps[:, GW:GW + HEADS])
                nc.sync.dma_start(out=adl[t * 128:(t + 1) * 128, :HEADS],
                                  in_=pa[:])

            def pack3(ps, t):
                s = slice_of_tile[t]
                r0 = (t - sl_t0[s]) * 128
                pk = epil.tile([128, GW3], dt.bfloat16, tag="pack3")
                nc.scalar.copy(out=pk[:], in_=ps[:, :GW3])
                nc.sync.dma_start(out=loc3s[s][r0:r0 + 128, :GW3], in_=pk[:])
                pa = epil.tile([128, 1], dt.bfloat16, tag="packa3")
                nc.scalar.copy(out=pa[:], in_=ps[:, GW3:GW3 + 1])
                nc.sync.dma_start(out=adl3[t * 128:(t + 1) * 128, :1],
                                  in_=pa[:])

            def h1_phase():
                # own-shard a_dst -> adl1
                for blk0 in range(0, ntiles, 8):
                    nt = min(8, ntiles - blk0)
                    xs2 = sbuf.tile([128, 8, 128], dt.bfloat16, tag="xs2")
                    nc.sync.dma_start(
                        out=xs2[:, :nt, :],
                        in_=xTb[:, blk0 * 128:(blk0 + nt) * 128].rearrange(
                            "p (k c) -> p k c", c=128))
                    pa8 = epil.tile([128, 8, HEADS], dt.bfloat16, tag="pa8")
                    for j in range(nt):
                        ps = psum_h.tile([128, PACK], dt.float32, tag="hps")
                        nc.tensor.matmul(ps[:, :HEADS], lhsT=xs2[:, j, :],
                                         rhs=rhs1a_s[:], start=True, stop=True)
                        nc.scalar.copy(out=pa8[:, j, :], in_=ps[:, :HEADS])
                    nc.sync.dma_start(
                        out=adl1[blk0 * 128:(blk0 + nt) * 128, :HEADS]
                        .rearrange("(k p) w -> p k w", p=128),
                        in_=pa8[:, :nt, :])
                # replicated [h | a_src] for all cores' tiles -> local tabs1
                for s in range(len(SLICES)):
                    for c in range(NCORES):
                        for b0 in range(0, SLICES[s], 7):
                            nt = min(7, SLICES[s] - b0)
                            col0 = c * nmax + (sl_t0[s] + b0) * 128
                            xsl = sbuf.tile([128, 7, 128], dt.bfloat16,
                                            tag="xsl")
                            nc.sync.dma_start(
                                out=xsl[:, :nt, :],
                                in_=xTf[:, col0:col0 + nt * 128].rearrange(
                                    "p (k c) -> p k c", c=128))
                            pk7 = epil.tile([128, 7, TWB], dt.uint8,
                                            tag="pk7")
                            for j in range(nt):
                                ps = psum_h.tile([128, PACK], dt.float32,
                                                 tag="hps")
                                nc.tensor.matmul(ps[:, :GW], lhsT=xsl[:, j, :],
                                                 rhs=rhs1b_s[:], start=True,
                                                 stop=True)
                                nc.scalar.copy(
                                    out=pk7[:, j, :H2].bitcast(dt.float8e4),
                                    in_=ps[:, :H2])
                                nc.scalar.copy(
                                    out=pk7[:, j, H2:H2 + 2 * HEADS].bitcast(
                                        dt.bfloat16),
                                    in_=ps[:, H2:H2 + HEADS])
                            r0 = c * sl_rows[s] + b0 * 128
                            nc.sync.dma_start(
                                out=tabs1[s][r0:r0 + nt * 128, :ROWB]
                                .rearrange("(k p) w -> p k w", p=128),
                                in_=pk7[:, :nt, :ROWB])

            def allgather(local, table_ap):
                nc.gpsimd.collective_compute(
                    "AllGather", Alu.bypass,
                    replica_groups=[list(range(NCORES))],
                    ins=[local[:].opt()], outs=[table_ap.opt()])

            def epilogue12(t, ps, rhs_next_s, b_s, layer):
                deneps = epil.tile([128, HEADS], dt.float32, tag="deneps")
                nc.vector.tensor_scalar_add(deneps[:], ps[:, H2:H2 + HEADS],
                                            1e-16)
                recip = epil.tile([128, HEADS], dt.float32, tag="recip")
                nc.vector.reciprocal(recip[:], deneps[:])
                act = epil.tile([128, H2], dt.float32, tag="act")
                nc.vector.tensor_tensor(
                    out=act[:].rearrange("p (c h) -> p c h", h=HEADS),
                    in0=ps[:, :H2].rearrange("p (c h) -> p c h", h=HEADS),
                    in1=recip[:].unsqueeze(1).to_broadcast([128, HID, HEADS]),
                    op=Alu.mult)
                nc.vector.tensor_add(out=act[:], in0=act[:], in1=b_s[:])
                nc.scalar.activation(out=act[:], in_=act[:], func=Act.Relu)
                w = PACK if layer == 1 else OUT_C + 2
                hps = psum_h.tile([128, PACK], dt.float32, tag="hps")
                for kc in range(2):
                    tp = psum_tp.tile([128, 128], dt.float32, tag="tp")
                    nc.tensor.transpose(
                        out=tp[:], in_=act[:, kc * 128:(kc + 1) * 128],
                        identity=ident[:])
                    aT = epil.tile([128, 128], dt.float32, tag="aT")
                    nc.scalar.copy(out=aT[:], in_=tp[:])
                    nc.tensor.matmul(
                        hps[:, :w], lhsT=aT[:],
                        rhs=rhs_next_s[:, kc * w:(kc + 1) * w],
                        start=(kc == 0), stop=(kc == 1))
                if layer == 1:
                    pack12(hps, loc2s, adl2, t)
                else:
                    pack3(hps, t)

            def epilogue3(t, ps):
                deneps = epil.tile([128, 1], dt.float32, tag="deneps3")
                nc.vector.tensor_scalar_add(deneps[:], ps[:, OUT_C:OUT_C + 1],
                                            1e-16)
                recip = epil.tile([128, 1], dt.float32, tag="recip3")
                nc.vector.reciprocal(recip[:], deneps[:])
                o3 = epil.tile([128, OUT_C], dt.float32, tag="o3")
                nc.vector.tensor_scalar(
                    out=o3[:], in0=ps[:, :OUT_C], scalar1=recip[:, :1],
                    scalar2=None, op0=Alu.mult)
                nc.vector.tensor_add(out=o3[:], in0=o3[:], in1=b3_s[:])
                mneg = epil.tile([128, 1], dt.float32, tag="mneg")
                nc.vector.tensor_reduce(
                    out=mneg[:], in_=o3[:], axis=mybir.AxisListType.X,
                    op=Alu.max, negate=True)
                es = epil.tile([128, OUT_C], dt.float32, tag="es")
                ssum = epil.tile([128, 1], dt.float32, tag="ssum")
                nc.scalar.activation(out=es[:], in_=o3[:], func=Act.Exp,
                                     bias=mneg[:, :1], accum_out=ssum[:, :1])
                lse = epil.tile([128, 1], dt.float32, tag="lse")
                nc.scalar.activation(out=lse[:], in_=ssum[:], func=Act.Ln)
                fin = epil.tile([128, OUT_C], dt.float32, tag="fin")
                nc.vector.tensor_scalar(
                    out=fin[:], in0=o3[:], scalar1=mneg[:, :1],
                    scalar2=lse[:, :1], op0=Alu.add, op1=Alu.subtract)
                nc.sync.dma_start(out=out[t * 128:(t + 1) * 128, :], in_=fin[:])

            import os as _os
            ABL = _os.environ.get("GAT_ABLATE", "")

            def aggregate(layer, tabs, adl, rhs_next_s, b_s, tabs_next=None):
                if layer == 3:
                    gw, nfeat, nh, tw = GW3, OUT_C, 1, TW3
                else:
                    gw, nfeat, nh, tw = GW, H2, HEADS, TWB
                locs_next = loc2s if layer == 1 else loc3s

                qrr = [0]  # round-robin SWDGE queue cursor

                def next_q():
                    q = qrr[0]
                    qrr[0] = (q + 1) % 4
                    return q

                ps_cur = None
                for sup in range(nsup):
                    i0 = sup * KSUP * 8

                    if layer == 3:
                        gt = gbuf.tile([128, KSUP, TW3], dt.bfloat16,
                                       tag="g3")
                    else:
                        gt = gbuf.tile([128, KSUP, TWB], dt.uint8, tag="g")
                    if "nogather" not in ABL:
                        for (a, b, tag) in runs_by_sup[sup]:
                            a0, b0 = a - sup * KSUP, b - sup * KSUP
                            nidx = (b - a) * 128
                            src_ap = tabs[tag][:]
                            nc.gpsimd.dma_gather(
                                out_ap=gt[:, a0:b0, :], in_ap=src_ap,
                                idxs_ap=sidx_all[:, i0 + a0 * 8:i0 + b0 * 8],
                                num_idxs=nidx, num_idxs_reg=nidx, elem_size=tw,
                                queue_num=next_q())
                    dts = gbuf.tile([128, KSUP, TW3], dt.bfloat16, tag="dts")
                    if "nogather" not in ABL:
                        nc.gpsimd.dma_gather(
                            out_ap=dts[:], in_ap=adl[:],
                            idxs_ap=didx_all[:, i0:i0 + KSUP * 8],
                            num_idxs=KSUP * 128, num_idxs_reg=KSUP * 128,
                            elem_size=TW3, queue_num=next_q())
                    if "nocompute" in ABL:
                        continue

                    if layer != 3:
                        asrc = gt[:, :, H2:H2 + 2 * HEADS].bitcast(
                            dt.bfloat16)
                    else:
                        asrc = gt[:, :, nfeat:nfeat + nh]
                    wt = gbuf.tile([128, KSUP, nh], dt.bfloat16, tag="wt")
                    nc.vector.tensor_tensor(
                        out=wt[:], in0=asrc, in1=dts[:, :, :nh], op=Alu.add)
                    lk = gbuf.tile([128, KSUP, nh], dt.bfloat16, tag="lk")
                    nc.vector.tensor_scalar_mul(lk[:], wt[:], NEG_SLOPE)
                    nc.vector.tensor_tensor(out=wt[:], in0=wt[:], in1=lk[:],
                                            op=Alu.max)
                    nc.scalar.activation(out=wt[:], in_=wt[:], func=Act.Exp)
                    if layer != 3:
                        gm = gbuf.tile([128, KSUP, GW], dt.bfloat16, tag="gm")
                        nc.vector.tensor_tensor(
                            out=gm[:, :, :nfeat].rearrange(
                                "p k (c h) -> p k c h", h=HEADS),
                            in0=gt[:, :, :nfeat].bitcast(
                                dt.float8e4).rearrange(
                                "p k (c h) -> p k c h", h=HEADS),
                            in1=wt[:].unsqueeze(2).to_broadcast(
                                [128, KSUP, HID, HEADS]),
                            op=Alu.mult)
                        nc.vector.tensor_copy(gm[:, :, nfeat:nfeat + nh],
                                              wt[:])
                    else:
                        gm = gt
                        nc.vector.tensor_tensor(
                            out=gm[:, :, :nfeat], in0=gt[:, :, :nfeat],
                            in1=wt[:].to_broadcast([128, KSUP, nfeat]),
                            op=Alu.mult)
                        nc.vector.tensor_copy(gm[:, :, nfeat:nfeat + nh],
                                              wt[:])

                    mt = mbuf.tile([128, KSUP * 128], dt.bfloat16, tag="mt")
                    for kk in range(KSUP):
                        q = sup * KSUP + kk
                        t = int(tile_of_chunk[q])
                        nc.vector.tensor_scalar(
                            out=mt[:, kk * 128:(kk + 1) * 128], in0=iota_s[:],
                            scalar1=seg_all[:, q:q + 1], scalar2=None,
                            op0=Alu.is_equal)
                        if q == first_chunk[t]:
                            ps_cur = psum_seg.tile([128, GW], dt.float32,
                                                   tag="segps")
                        nc.tensor.matmul(
                            ps_cur[:, :gw],
                            lhsT=mt[:, kk * 128:(kk + 1) * 128],
                            rhs=gm[:, kk, :gw],
                            start=(q == first_chunk[t]),
                            stop=(q == last_chunk[t]))
                        if q == last_chunk[t] and "noepi" not in ABL:
                            if layer == 3:
                                epilogue3(t, ps_cur)
                            else:
                                epilogue12(t, ps_cur, rhs_next_s, b_s, layer)
                            s = slice_of_tile[t]
                            if (tabs_next is not None
                                    and t == sl_last_tile[s]):
                                allgather(locs_next[s], tabs_next[s][:])

            import os
            nphase = int(os.environ.get("GAT_PHASES", "3"))
            for _rep in range(repeat):
                tabs2 = [dram.tile([NCORES * r, TWB], dt.uint8,
                                   addr_space="Shared",
                                   name=f"tab2_{s}_{_rep}")
                         for s, r in enumerate(sl_rows)]
                tabs3 = [dram.tile([NCORES * r, TW3], dt.bfloat16,
                                   addr_space="Shared",
                                   name=f"tab3_{s}_{_rep}")
                         for s, r in enumerate(sl_rows)]
                h1_phase()
                if nphase >= 1:
                    aggregate(1, tabs1, adl1, rhs2_s, b1_s, tabs_next=tabs2)
                if nphase >= 2:
                    aggregate(2, tabs2, adl2, rhs3_s, b2_s, tabs_next=tabs3)
                if nphase >= 3:
                    aggregate(3, tabs3, adl3, None, None)

    nc.compile()
    return nc


_CACHE = {}


def make_in_maps(g, x, weights):
    rhs1b, rhs1a, rhs2, rhs3, b1r, b2r, b3r = weights
    npc, nmax = g["npc"], g["nmax"]
    x = np.asarray(x, np.float32)
    iota = np.tile(np.arange(128, dtype=np.float32)[None, :],
                   (128, 1)).astype(_BF16)
    xTf = np.zeros((IN_C, NCORES * nmax), _BF16)
    for k in range(NCORES):
        xTf[:, k * nmax:k * nmax + npc] = x[k * npc:(k + 1) * npc].T
    in_maps = []
    for k in range(NCORES):
        in_maps.append({
            "xTf": xTf, "xTb": xTf[:, k * nmax:(k + 1) * nmax],
            "rhs1b": rhs1b, "rhs1a": rhs1a, "rhs2": rhs2, "rhs3": rhs3,
            "b1r": b1r, "b2r": b2r, "b3r": b3r, "iota": iota,
            "src_w": g["src_w"][k], "adst_w": g["adst_w"][k],
            "seg": g["seg"][k],
        })
    return in_maps


def kernel(x, edge_index, W1, as1, ad1, b1, W2, as2, ad2, b2, W3, as3, ad3, b3,
           _repeat=1):
    from concourse.bass_utils import run_bass_kernel_spmd

    x = np.asarray(x, np.float32)
    edge_index = np.asarray(edge_index)
    g = _prep_graph(edge_index)
    weights = _prep_weights(
        W1, as1, ad1, b1, W2, as2, ad2, b2, W3, as3, ad3, b3)

    key = (hash(edge_index.tobytes()), _repeat)
    if key not in _CACHE:
        _CACHE[key] = _build_bass(g, repeat=_repeat)
    nc = _CACHE[key]

    in_maps = make_in_maps(g, x, weights)
    res = run_bass_kernel_spmd(nc, in_maps, core_ids=list(range(NCORES)))
    npc = g["npc"]
    outf = np.zeros((N, OUT_C), np.float32)
    for k in range(NCORES):
        outf[k * npc:(k + 1) * npc] = res.results[k]["out"][:npc]
    return outf

